# revision 17
# baseline (speedup 1.0000x reference)
"""MixLinear GEMM kernel for Trainium2 (8 NeuronCores, column-parallel).

Computes, for full inputs:
    inputs = x.reshape(-1, 4096)
    act_outliers = inputs[:, ind]
    inputs_z = inputs with ind-columns zeroed
    x_scale = clamp(rowmax(|inputs_z|)/127, 1e-8)
    q_x = round(inputs_z / x_scale)                  (|q_x| <= 127 by construction)
    y = (q_x @ q_weight.T) * x_scale * scale_col + act_outliers @ weight_cache.T + bias

Sharding: q_weight/scale_col/weight_cache/bias are sharded along out_features
across the 8 cores (column parallel); x/ind are replicated. Each core produces
its (512, 1376) output shard; the host concatenates.
"""

import os
import sys

import numpy as np

sys.path.insert(0, "/opt/trn_rl_repo")

import concourse.bass as bass  # noqa: E402
import concourse.mybir as mybir  # noqa: E402
import concourse.tile as tile  # noqa: E402
from concourse import bacc  # noqa: E402

N_CORES = 8
M = 512  # 8*64 rows
K = 4096  # in_features
OUT = 11008  # out_features
OSH = OUT // N_CORES  # 1376 per-core shard
FP = 256  # outlier columns
KT = K // 128  # 32 k-tiles
MT = M // 128  # 4 m-tiles
OTILES = (OSH + 127) // 128  # 11 o-tiles (last is 96 rows)
OPAD = OTILES * 128  # 1408
MAGIC = 1536.0  # fp16 spacing is 1.0 in [1024, 2048): forces round-to-int
O_CHUNK = 512  # moving-operand free width for the main GEMM
XH = 2048  # x streamed in half-tiles to save SBUF
XQ = 1024  # absmax computed in chunks of this width (small scratch)

f32 = mybir.dt.float32
f16 = mybir.dt.float16
bf16 = mybir.dt.bfloat16
i32 = mybir.dt.int32
Alu = mybir.AluOpType
Act = mybir.ActivationFunctionType


def build_program():
    nc = bacc.Bacc(
        "TRN2", target_bir_lowering=False, debug=False, num_devices=N_CORES
    )

    x_d = nc.dram_tensor("x_in", [M, K], f32, kind="ExternalInput").ap()
    w_d = nc.dram_tensor("w_in", [OSH, K], i32, kind="ExternalInput").ap()
    cache_d = nc.dram_tensor("cache_in", [OSH, FP], f32, kind="ExternalInput").ap()
    mask_d = nc.dram_tensor("mask_in", [1, K], bf16, kind="ExternalInput").ap()
    maskcol_d = nc.dram_tensor("maskcol_in", [128, KT], f32, kind="ExternalInput").ap()
    ind_d = nc.dram_tensor("ind_in", [1, FP], i32, kind="ExternalInput").ap()
    biascol_d = nc.dram_tensor("biascol_in", [128, OTILES], f32, kind="ExternalInput").ap()
    sccol_d = nc.dram_tensor("sccol_in", [128, OTILES], f32, kind="ExternalInput").ap()
    scrow_d = nc.dram_tensor("scrow_in", [1, OSH], f32, kind="ExternalInput").ap()
    y_d = nc.dram_tensor("y_out", [M, OSH], f32, kind="ExternalOutput").ap()

    with tile.TileContext(nc) as tc:
        with (
            tc.tile_pool(name="persist", bufs=1) as persist,
            tc.tile_pool(name="xpool", bufs=3) as xpool,
            tc.tile_pool(name="xzpool", bufs=1) as xzpool,
            tc.tile_pool(name="qnpool", bufs=2) as qnpool,
            tc.tile_pool(name="s2pool", bufs=2) as s2pool,
            tc.tile_pool(name="cachepool", bufs=2) as cachepool,
            tc.tile_pool(name="wnpool", bufs=2) as wnpool,
            tc.tile_pool(name="wtpool", bufs=2) as wtpool,
            tc.tile_pool(name="ypool", bufs=3) as ypool,
            tc.tile_pool(name="psg", bufs=1, space="PSUM") as psg,
            tc.tile_pool(name="psr", bufs=1, space="PSUM") as psr,
            tc.tile_pool(name="psmain", bufs=4, space="PSUM") as psmain,
        ):
            # ---------- persistent tiles ----------
            q_offT = persist.tile([128, KT, M], f16)  # 4 MB: q^T (k-part, kk, m)
            mask_col = persist.tile([128, KT], f32)
            mask_bc = persist.tile([128, K], bf16)  # mask broadcast across partitions
            ind_bc_i = persist.tile([128, FP], i32)
            ind_bc = persist.tile([128, FP], f32)
            iota_col = persist.tile([128, KT], f32)
            identity = persist.tile([128, 128], f32)
            recip_row = persist.tile([1, M], f16)
            cacheT = persist.tile([128, 3, OSH], f16)  # (j-part, j-chunk, o)
            actT = persist.tile([128, 2, M], f16)  # outlier activations^T
            sc_bc = persist.tile([128, OSH], f32)
            am_all = persist.tile([128, MT], f32)
            am_parts = persist.tile([128, MT * (K // XQ)], f32)
            xs_all = persist.tile([128, MT], f32)
            recip_all = persist.tile([128, MT], f32)
            sc_sb = persist.tile([128, OTILES], f32)
            bias_sb = persist.tile([128, OTILES], f32)
            recipsc = persist.tile([128, OTILES], f32)
            biasp = persist.tile([128, OTILES], f32)

            # ---------- small setup ----------
            nc.sync.dma_start(out=mask_col, in_=maskcol_d)
            nc.sync.dma_start(out=sc_sb, in_=sccol_d)
            nc.sync.dma_start(out=bias_sb, in_=biascol_d)
            # broadcasts across partitions: DRAM AP with partition-step 0
            nc.gpsimd.dma_start(
                out=mask_bc,
                in_=bass.AP(mask_d.tensor, mask_d.offset, [[0, 128], [1, K]]),
            )
            nc.gpsimd.dma_start(
                out=ind_bc_i,
                in_=bass.AP(ind_d.tensor, ind_d.offset, [[0, 128], [1, FP]]),
            )
            nc.vector.tensor_copy(ind_bc, ind_bc_i)
            nc.gpsimd.dma_start(
                out=sc_bc,
                in_=bass.AP(scrow_d.tensor, scrow_d.offset, [[0, 128], [1, OSH]]),
            )
            # iota_col[p, kk] = kk*128 + p
            nc.gpsimd.iota(
                iota_col,
                pattern=[[128, KT]],
                base=0,
                channel_multiplier=1,
                allow_small_or_imprecise_dtypes=True,
            )
            # identity[p, f] = (p == f)
            nc.gpsimd.memset(identity, 1.0)
            nc.gpsimd.affine_select(
                out=identity,
                in_=identity,
                compare_op=Alu.is_equal,
                fill=0.0,
                base=0,
                pattern=[[-1, 128]],
                channel_multiplier=1,
            )

            # scale prep: 1/scale_col and bias/scale_col  (padded sc entries are 1.0)
            nc.vector.reciprocal(out=recipsc, in_=sc_sb)
            nc.vector.tensor_tensor(
                out=biasp, in0=bias_sb, in1=recipsc, op=Alu.mult
            )

            # ---------- phase 1: quantization (natural layout) ----------
            nhalf = K // XH  # 2
            nq = XH // XQ  # 2
            for mt in range(MT):
                ms = slice(mt * 128, (mt + 1) * 128)
                x_hs = []
                for h in range(nhalf):
                    x_h = xpool.tile([128, XH], f32, tag="x", name=f"x_{mt}_{h}")
                    nc.sync.dma_start(
                        out=x_h, in_=x_d[ms, h * XH : (h + 1) * XH]
                    )
                    x_hs.append(x_h)
                    for q in range(nq):
                        xz = xzpool.tile([128, XQ], f32, tag="xz")
                        nc.vector.tensor_tensor(
                            out=xz,
                            in0=x_h[:, q * XQ : (q + 1) * XQ],
                            in1=mask_bc[:, (h * nq + q) * XQ : (h * nq + q + 1) * XQ],
                            op=Alu.mult,
                        )
                        pcol = mt * (K // XQ) + h * nq + q
                        nc.vector.tensor_reduce(
                            out=am_parts[:, pcol : pcol + 1],
                            in_=xz,
                            axis=mybir.AxisListType.X,
                            op=Alu.max,
                            apply_absolute_value=True,
                        )
                nc.vector.tensor_reduce(
                    out=am_all[:, mt : mt + 1],
                    in_=am_parts[:, mt * (K // XQ) : (mt + 1) * (K // XQ)],
                    axis=mybir.AxisListType.X,
                    op=Alu.max,
                    apply_absolute_value=False,
                )
                # xs = max(absmax/127, 1e-8); recip = 1/xs
                nc.vector.tensor_scalar(
                    xs_all[:, mt : mt + 1],
                    am_all[:, mt : mt + 1],
                    1.0 / 127.0,
                    1e-8,
                    Alu.mult,
                    Alu.max,
                )
                nc.vector.reciprocal(
                    out=recip_all[:, mt : mt + 1], in_=xs_all[:, mt : mt + 1]
                )
                for h in range(nhalf):
                    # q_off = x*recip + 1536 -> fp16 write rounds to int (RNE)
                    qn = qnpool.tile([128, XH], f16, tag="qn", name=f"qn_{mt}_{h}")
                    nc.scalar.activation(
                        out=qn,
                        in_=x_hs[h],
                        func=Act.Copy,
                        bias=MAGIC,
                        scale=recip_all[:, mt : mt + 1],
                    )
                    # transpose into q_offT[:, k-half, m-slice]
                    nc.sync.dma_start(
                        out=q_offT[:, h * (XH // 128) : (h + 1) * (XH // 128), ms],
                        in_=qn,
                        transpose=True,
                    )

            # recip_row[0, m] = recip[m] via PE transpose (lhsT.T @ identity)
            for mt in range(MT):
                ps_r = psr.tile([1, 128], f32, tag="psr")
                nc.tensor.matmul(
                    ps_r, lhsT=recip_all[:, mt : mt + 1], rhs=identity,
                    start=True, stop=True,
                )
                nc.vector.tensor_copy(
                    recip_row[0:1, mt * 128 : (mt + 1) * 128], ps_r
                )

            # ---------- phase 1.5: gather outliers via one-hot matmuls ----------
            ps_gs = []
            for jc in range(2):
                ps_g = psg.tile([128, M], f32, tag=f"psg{jc}", name=f"psg{jc}")
                ps_gs.append(ps_g)
            for kk in range(KT):
                s2 = s2pool.tile([128, FP], f16, tag="s2")
                nc.vector.tensor_scalar(
                    s2, ind_bc, iota_col[:, kk : kk + 1], None, Alu.is_equal
                )
                for jc in range(2):
                    nc.tensor.matmul(
                        ps_gs[jc],
                        lhsT=s2[:, jc * 128 : (jc + 1) * 128],
                        rhs=q_offT[:, kk, :],
                        start=(kk == 0),
                        stop=(kk == KT - 1),
                    )
            for jc in range(2):
                # actT' = gathered q_off - 1536  (= x[.,ind]*recip, with +-0.5 quant err)
                nc.vector.tensor_scalar(
                    actT[:, jc, :], ps_gs[jc], -MAGIC, None, Alu.add
                )

            # masked, unbiased q^T in place: q = (q_off - 1536) * mask
            for kk in range(KT):
                nc.vector.tensor_scalar(
                    q_offT[:, kk, :],
                    q_offT[:, kk, :],
                    MAGIC,
                    mask_col[:, kk : kk + 1],
                    Alu.subtract,
                    Alu.mult,
                )

            # ---------- phase 2a: outlier weights (cache/sc, bias/sc) -> cacheT ----------
            for ot in range(OTILES):
                osz = min(128, OSH - ot * 128)
                cext = cachepool.tile([128, 3 * 128], f16, tag="cext")
                nc.gpsimd.memset(cext, 0.0)
                cn = cachepool.tile([128, FP], f32, tag="cn")
                nc.sync.dma_start(
                    out=cn[:osz], in_=cache_d[ot * 128 : ot * 128 + osz, :]
                )
                nc.vector.tensor_scalar(
                    cext[:osz, 0:FP],
                    cn[:osz],
                    recipsc[:osz, ot : ot + 1],
                    None,
                    Alu.mult,
                )
                nc.vector.tensor_copy(
                    cext[:osz, FP : FP + 1], biasp[:osz, ot : ot + 1]
                )
                nc.sync.dma_start(
                    out=cacheT[:, :, ot * 128 : ot * 128 + osz],
                    in_=cext[:osz, :],
                    transpose=True,
                )

            # ---------- phase 2b: main GEMM over o-chunks ----------
            nch = (OSH + O_CHUNK - 1) // O_CHUNK
            for c in range(nch):
                o0 = c * O_CHUNK
                cw = min(O_CHUNK, OSH - o0)
                wt = wtpool.tile([128, KT, O_CHUNK], f16, tag="wt")
                for i in range((cw + 127) // 128):
                    osz = min(128, cw - i * 128)
                    wn = wnpool.tile([128, K], f16, tag="wn")
                    # SWDGE cast DMA: int32 -> fp16 (values are int8-ranged)
                    nc.gpsimd.dma_start(
                        out=wn[:osz],
                        in_=w_d[o0 + i * 128 : o0 + i * 128 + osz, :],
                    )
                    nc.sync.dma_start(
                        out=wt[:, :, i * 128 : i * 128 + osz],
                        in_=wn[:osz],
                        transpose=True,
                    )
                for mt in range(MT):
                    ms = slice(mt * 128, (mt + 1) * 128)
                    ps = psmain.tile([128, O_CHUNK], f32, tag="ps")
                    for kk in range(KT):
                        nc.tensor.matmul(
                            ps[:, :cw],
                            lhsT=q_offT[:, kk, ms],
                            rhs=wt[:, kk, :cw],
                            start=(kk == 0),
                            stop=False,
                        )
                    for jc in range(2):
                        nc.tensor.matmul(
                            ps[:, :cw],
                            lhsT=actT[:, jc, ms],
                            rhs=cacheT[:, jc, o0 : o0 + cw],
                            start=False,
                            stop=False,
                        )
                    nc.tensor.matmul(
                        ps[:, :cw],
                        lhsT=recip_row[0:1, ms],
                        rhs=cacheT[0:1, 2, o0 : o0 + cw],
                        start=False,
                        stop=True,
                    )
                    ysb = ypool.tile([128, O_CHUNK], f32, tag="ysb")
                    nc.vector.scalar_tensor_tensor(
                        out=ysb[:, :cw],
                        in0=ps[:, :cw],
                        scalar=xs_all[:, mt : mt + 1],
                        in1=sc_bc[:, o0 : o0 + cw],
                        op0=Alu.mult,
                        op1=Alu.mult,
                    )
                    nc.sync.dma_start(out=y_d[ms, o0 : o0 + cw], in_=ysb[:, :cw])

    nc.compile()
    return nc


_NC_CACHE = None


def get_program():
    global _NC_CACHE
    if _NC_CACHE is None:
        _NC_CACHE = build_program()
    return _NC_CACHE


def make_in_maps(x, q_weight, scale_col, weight_cache, ind, bias):
    x2 = np.ascontiguousarray(np.asarray(x, dtype=np.float32).reshape(M, K))
    q_weight = np.asarray(q_weight, dtype=np.int32)
    scale_col = np.asarray(scale_col, dtype=np.float32).reshape(OUT)
    weight_cache = np.asarray(weight_cache, dtype=np.float32)
    ind_np = np.asarray(ind, dtype=np.int32).reshape(FP)
    bias_np = np.asarray(bias, dtype=np.float32).reshape(OUT)

    import ml_dtypes

    mask = np.ones(K, dtype=np.float32)
    mask[ind_np] = 0.0
    mask_bf = mask.astype(ml_dtypes.bfloat16).reshape(1, K)
    maskcol = np.ascontiguousarray(mask.reshape(KT, 128).T)  # [p, kk]
    ind_in = ind_np.reshape(1, FP)

    in_maps = []
    for c in range(N_CORES):
        sl = slice(c * OSH, (c + 1) * OSH)
        sc_sh = scale_col[sl]
        bias_sh = bias_np[sl]
        sc_pad = np.full(OPAD, 1.0, dtype=np.float32)
        sc_pad[:OSH] = sc_sh
        bias_pad = np.zeros(OPAD, dtype=np.float32)
        bias_pad[:OSH] = bias_sh
        in_maps.append(
            {
                "x_in": x2,
                "w_in": np.ascontiguousarray(q_weight[sl]),
                "cache_in": np.ascontiguousarray(weight_cache[sl]),
                "mask_in": mask_bf,
                "maskcol_in": maskcol,
                "ind_in": ind_in,
                "biascol_in": np.ascontiguousarray(bias_pad.reshape(OTILES, 128).T),
                "sccol_in": np.ascontiguousarray(sc_pad.reshape(OTILES, 128).T),
                "scrow_in": sc_sh.reshape(1, OSH),
            }
        )
    return in_maps


def kernel(x, q_weight, scale_col, weight_cache, ind, bias):
    from concourse.bass_utils import run_bass_kernel_spmd

    nc = get_program()
    in_maps = make_in_maps(x, q_weight, scale_col, weight_cache, ind, bias)
    res = run_bass_kernel_spmd(nc, in_maps, core_ids=list(range(N_CORES)))
    shards = [res.results[c]["y_out"] for c in range(N_CORES)]
    y = np.concatenate(shards, axis=1)
    return y.reshape(8, 64, OUT).astype(np.float32)
